# revision 27
# baseline (speedup 1.0000x reference)
"""Trainium2 Bass kernel for CartesianDecomposedAttention (complex-valued attention).

Reference math (complex):
  Q = (x @ wq.T) * rotor ; K = (x @ wk.T) * rotor ; V = x @ wv.T
  scores = Q conj(K)^T / sqrt(Dh)
  attn_w = softmax(scores.re) * exp(i * scores.im)
  out    = (attn_w @ V) @ wo.T        -> stack([re, im], -1)

Sharding over 8 cores: core c -> batch b=c//4, head-group g=c%4 (4 heads, 256
model dims per group). Each core computes a partial output [S, D] (re+im);
the host sums the 4 group partials per batch (no on-device collectives).

Device decomposition (matmul operands fp16, PSUM accumulation fp32):
  - Component-STACKED score/AV matmuls: per head, Q/K live as [128, S] tiles
    with re on partitions 0:64 and im on 64:128, so each score matmul is a
    single full-rate k=128 op:
      q_s(h)  = [Qr'; Qi']      k_s1(h) = [Kr'; Ki']    k_s2(h) = [-Ki'; Kr']
      Sr  = k_s1^T q_s          Sip = k_s2^T q_s        (scoresT [t, s])
    AV uses the V tile's component pairs as stacked lhsT columns
    (comps (Vr,Vi) -> rows 0:64 += ar*Vr, rows 64:128 += ar*Vi; comps
    (-Vi,Vr) with rhs=ai), producing at once the stacked [re;im] attn out.
    The stacked attn feeds the out-projection with host-stacked wo:
      woS_re(h) = [woT_re(h); -woT_im(h)]   woS_im(h) = [woT_im(h); woT_re(h)]
  - Q/K stacks are assembled from the RoPE'd pair tiles by SBUF->SBUF DMAs
    ([64, S] halves; partition-crossing moves are free on the DMA engines).
  - RoPE via host-built cos/sin tables [128, S]; the 1/sqrt(Dh) score
    scale is folded into the Q-side tables.
  - softmax without max-subtraction (scores in [-8, 8]); the denominator is a
    ones-vector matmul over exp tiles, applied *after* the AV matmul
    (reciprocal on a [1,512] row, partition-broadcast AP view in the mul).
  - HW Sin is valid only on [-pi, pi]: two add_range_wraps straight from the
    scores.im PSUM tile (shift 0 -> sin arg, shift pi/2 -> cos arg).
  - ACT exp and sin live in different table sets (~2.7us per switch): chunks
    alternate exp-block / sin-block with explicit scheduling deps.
  - Pipelined emission: pair-1 Q projection is emitted before chunk (0,0) so
    the PE has independent work while the first stack DMAs land; the output
    projection for s<512 is emitted mid-stream (after chunks (0,0),(1,0))
    to keep the tail short.
"""

import sys

for _p in ("/opt/trn_rl_repo",):
    if _p not in sys.path:
        sys.path.insert(0, _p)

import numpy as np
from contextlib import ExitStack

import concourse.bass as bass
import concourse.tile as tile
from concourse import bacc, mybir
from concourse.bass_utils import run_bass_kernel_spmd
from concourse.tile_rust import add_dep_helper

F32 = mybir.dt.float32
MM_DT = mybir.dt.float16          # matmul operand dtype
MM_NP = np.float16                # host-side dtype for matmul operands
TAB_DT = mybir.dt.float16         # rope table dtype
ALU = mybir.AluOpType
ACTF = mybir.ActivationFunctionType

D = 1024          # model dim
S = 1024          # sequence length
DH = 64           # head dim
JG = 256          # j-columns per head group (4 heads)
KT = 8            # k-tiles of 128 over D
PI = float(np.pi)


def _dep(frm, to, reason):
    """Scheduling-order dependency: `to` must come after `frm`.

    add_dep_helper's arg order is (waiter, prerequisite).
    """
    add_dep_helper(to.ins, frm.ins, sync=False, reason=reason)


def _build_kernel(tc, ins, outs):
    nc = tc.nc
    ctx = ExitStack()

    persist = ctx.enter_context(tc.tile_pool(name="persist", bufs=1))
    # shared PSUM pools: "mm" serves QKV projections, scores and the output
    # projection; "at" the AV accumulators; "r" the softmax denominators.
    pmm = ctx.enter_context(tc.tile_pool(name="ps_mm", bufs=5, space="PSUM"))
    pat = ctx.enter_context(tc.tile_pool(name="ps_at", bufs=2, space="PSUM"))
    ppr = ctx.enter_context(tc.tile_pool(name="ps_r", bufs=1, space="PSUM"))

    # --- persistent tensors (span phases) ---
    # component-stacked per-head tiles: partitions 0:64 = re, 64:128 = im
    q_s = persist.tile([128, 4, S], MM_DT, name="q_s")     # [comp-stack, head, s]
    k_s1 = persist.tile([128, 4, S], MM_DT, name="k_s1")   # [Kr'; Ki']
    k_s2 = persist.tile([128, 4, S], MM_DT, name="k_s2")   # [-Ki'; Kr']
    # per-head-major comps (-Vi, Vr, Vi): slices [h, 0:2]=(-Vi,Vr) and
    # [h, 1:3]=(Vr,Vi) are contiguous 128-col stationary operands for AV
    v = persist.tile([128, KT, 4, 3, 64], MM_DT, name="v")
    attn_s = persist.tile([128, 4, S], MM_DT, name="attn_s")  # [re|im dims, head, s]
    ones_col = persist.tile([128, 1], MM_DT, name="ones_col")
    nc.vector.memset(ones_col, 1.0)
    # HW Sin is exact on [-pi,pi] and degrades smoothly beyond (~6e-3 abs err
    # at 1.25pi). Wrapping scores.im once with a +pi/4 shift centers the two
    # trig args (sin: W-pi/4, cos: W+pi/4) so both stay within +-1.25pi --
    # good enough at tol 2e-2 and it skips the second range-wrap entirely.
    pi4 = persist.tile([128, 1], F32, name="pi4")
    nc.vector.memset(pi4, PI / 4.0)
    pi4n = persist.tile([128, 1], F32, name="pi4n")
    nc.vector.memset(pi4n, -PI / 4.0)

    # phase-B pools (outer so they survive until the end of attention)
    pB = ExitStack()
    pE = pB.enter_context(tc.tile_pool(name="phB_E", bufs=4))
    pW = pB.enter_context(tc.tile_pool(name="phB_W", bufs=4))
    psm = pB.enter_context(tc.tile_pool(name="phB_sm", bufs=2))
    pbt = pB.enter_context(tc.tile_pool(name="phB_tmp", bufs=4))

    # phase-A pools (inner; released after the V projection to make room)
    phA = ExitStack()
    pa = phA.enter_context(tc.tile_pool(name="phA", bufs=1))
    pw = phA.enter_context(tc.tile_pool(name="phA_w", bufs=2))
    pt = phA.enter_context(tc.tile_pool(name="phA_tmp", bufs=6))
    pst = phA.enter_context(tc.tile_pool(name="phA_st", bufs=2))

    x_re = pa.tile([128, KT, S], MM_DT, name="x_re", tag="x_re")
    x_im = pa.tile([128, KT, S], MM_DT, name="x_im", tag="x_im")
    # chunked + interleaved so the first projection matmuls (k-tile 0 of both
    # components) start after ~1/4 of the transfer instead of all of it
    xsr = ins["xT_re"].rearrange("(kt p) s -> p kt s", p=128)
    xsi = ins["xT_im"].rearrange("(kt p) s -> p kt s", p=128)
    # x components on separate trigger queues so the feeds run in parallel.
    # s-half 0 first at kt granularity: the first projection's st=0 matmuls
    # need (kt, s<512) slices only, so the PE can start after ~128KB.
    for kt in range(KT):
        nc.sync.dma_start(out=x_re[:, kt, 0:512], in_=xsr[:, kt, 0:512])
        nc.scalar.dma_start(out=x_im[:, kt, 0:512], in_=xsi[:, kt, 0:512])
    for ksl in (slice(0, 2), slice(2, 4), slice(4, 6), slice(6, 8)):
        nc.sync.dma_start(out=x_re[:, ksl, 512:1024], in_=xsr[:, ksl, 512:1024])
        nc.scalar.dma_start(out=x_im[:, ksl, 512:1024], in_=xsi[:, ksl, 512:1024])

    tabs = {}
    for t in ("qc8", "qs8", "kcos", "ksin"):
        tt = pa.tile([128, S], TAB_DT, name=f"tab_{t}", tag=f"tab_{t}")
        nc.scalar.dma_start(out=tt, in_=ins[t])
        tabs[t] = tt

    def load_wv():
        # emitted after the first Q/K weight loads so it doesn't delay them
        wv = pw.tile([128, KT, 3, 256], MM_DT, name="wv", tag="wv", bufs=1)
        # comps in free dim: 0=T_imn, 1=T_re, 2=T_im so that
        #   rhs1 = comps[1:3] = [re | im]   (with lhsT = x_re)
        #   rhs2 = comps[0:2] = [imn | re]  (with lhsT = x_im)
        for ci, sfx in ((0, "P_imn"), (1, "P_re"), (2, "P_im")):
            nc.gpsimd.dma_start(out=wv[:, :, ci, :], in_=ins["wv" + sfx])
        return wv

    def emit_qk(wname, pair, ctab, stab):
        """Load one pair's weight slices, project, RoPE, assemble stacks.

        RoPE'd pair tiles ([128, S]: rows 0:64 head h0, 64:128 head h1) are
        scattered into the per-head component-stacked tiles via SBUF->SBUF
        DMAs ([64, S] halves each).
        """
        w_re = pw.tile([128, KT, 128], MM_DT, name=f"{wname}{pair}_re", tag="w_re")
        w_im = pw.tile([128, KT, 128], MM_DT, name=f"{wname}{pair}_im", tag="w_im")
        w_imn = pw.tile([128, KT, 128], MM_DT, name=f"{wname}{pair}_imn", tag="w_imn")
        # all three comps' kt 0:2 chunks first so the projection's first
        # matmuls (which need re+im+imn at kt0) start as soon as possible
        wsrc = ((w_re, "P_re"), (w_im, "P_im"), (w_imn, "P_imn"))
        for wt, sfx in wsrc:
            nc.gpsimd.dma_start(out=wt[:, 0:2, :],
                                in_=ins[wname + sfx][pair][:, 0:2, :])
        for wt, sfx in wsrc:
            nc.gpsimd.dma_start(out=wt[:, 2:8, :],
                                in_=ins[wname + sfx][pair][:, 2:8, :])
        if wname == "wq":
            r_t = pst.tile([128, S], MM_DT, name="qr_t", tag="st_r")
            i_t = pst.tile([128, S], MM_DT, name="qi_t", tag="st_i")
            in_t = None
        else:
            r_t = pst.tile([128, S], MM_DT, name="kr_t", tag="st_r")
            i_t = pst.tile([128, S], MM_DT, name="ki_t", tag="st_i")
            in_t = pst.tile([128, S], MM_DT, name="kin_t", tag="st_in")
        for st in range(2):
            ssl = slice(st * 512, st * 512 + 512)
            ps_r = pmm.tile([128, 512], F32, name="ps_r", tag="mm")
            ps_i = pmm.tile([128, 512], F32, name="ps_i", tag="mm")
            for kt in range(KT):
                xr = x_re[:, kt, ssl]
                xi = x_im[:, kt, ssl]
                nc.tensor.matmul(ps_r, lhsT=w_re[:, kt, :], rhs=xr,
                                 start=(kt == 0), stop=False)
                nc.tensor.matmul(ps_i, lhsT=w_re[:, kt, :], rhs=xi,
                                 start=(kt == 0), stop=False)
                nc.tensor.matmul(ps_r, lhsT=w_imn[:, kt, :], rhs=xi,
                                 start=False, stop=(kt == KT - 1))
                nc.tensor.matmul(ps_i, lhsT=w_im[:, kt, :], rhs=xr,
                                 start=False, stop=(kt == KT - 1))
            # RoPE products: p1=Tr*c p2=Ti*s p3=Tr*s p4=Ti*c
            ct = tabs[ctab][:, ssl]
            st_t = tabs[stab][:, ssl]
            p1 = pt.tile([128, 512], MM_DT, name="p1", tag="ropetmp")
            p2 = pt.tile([128, 512], MM_DT, name="p2", tag="ropetmp")
            p3 = pt.tile([128, 512], MM_DT, name="p3", tag="ropetmp")
            p4 = pt.tile([128, 512], MM_DT, name="p4", tag="ropetmp")
            nc.vector.tensor_mul(p1, ps_r, ct)
            nc.vector.tensor_mul(p2, ps_i, st_t)
            nc.vector.tensor_mul(p3, ps_r, st_t)
            nc.vector.tensor_mul(p4, ps_i, ct)
            nc.vector.tensor_sub(r_t[:, ssl], p1, p2)    # T'r = p1 - p2
            nc.vector.tensor_add(i_t[:, ssl], p3, p4)    # T'i = p3 + p4
            if in_t is not None:
                nc.vector.scalar_tensor_tensor(
                    in_t[:, ssl], in0=p3, scalar=-1.0, in1=p4,
                    op0=ALU.mult, op1=ALU.subtract)      # -T'i
        # scatter the pair tiles into the per-head stacks (partition moves)
        h0, h1 = pair * 2, pair * 2 + 1
        if wname == "wq":
            nc.sync.dma_start(out=q_s[0:64, h0, :], in_=r_t[0:64, :])
            nc.sync.dma_start(out=q_s[64:128, h0, :], in_=i_t[0:64, :])
            nc.sync.dma_start(out=q_s[0:64, h1, :], in_=r_t[64:128, :])
            nc.sync.dma_start(out=q_s[64:128, h1, :], in_=i_t[64:128, :])
        else:
            nc.sync.dma_start(out=k_s1[0:64, h0, :], in_=r_t[0:64, :])
            nc.sync.dma_start(out=k_s1[64:128, h0, :], in_=i_t[0:64, :])
            nc.sync.dma_start(out=k_s1[0:64, h1, :], in_=r_t[64:128, :])
            nc.sync.dma_start(out=k_s1[64:128, h1, :], in_=i_t[64:128, :])
            nc.sync.dma_start(out=k_s2[0:64, h0, :], in_=in_t[0:64, :])
            nc.sync.dma_start(out=k_s2[64:128, h0, :], in_=r_t[0:64, :])
            nc.sync.dma_start(out=k_s2[0:64, h1, :], in_=in_t[64:128, :])
            nc.sync.dma_start(out=k_s2[64:128, h1, :], in_=r_t[64:128, :])

    def emit_v(wv):
        for tblk in range(KT):
            ps_v = pmm.tile([128, 512], F32, name="ps_v", tag="mm")
            for kt in range(KT):
                lx_re = x_re[:, kt, tblk * 128:(tblk + 1) * 128]
                lx_im = x_im[:, kt, tblk * 128:(tblk + 1) * 128]
                nc.tensor.matmul(ps_v, lhsT=lx_re, rhs=wv[:, kt, 1:3, :],
                                 start=(kt == 0), stop=False)
                nc.tensor.matmul(ps_v, lhsT=lx_im, rhs=wv[:, kt, 0:2, :],
                                 start=False, stop=(kt == KT - 1))
            # copy out (ACT, strided over heads): comps (0: -Vi, 1: Vr, 2: Vi)
            nc.scalar.copy(v[:, tblk, :, 1, :], ps_v[:, 0:256])
            nc.scalar.copy(v[:, tblk, :, 2, :], ps_v[:, 256:512])
            nc.scalar.activation(v[:, tblk, :, 0, :], ps_v[:, 256:512],
                                 ACTF.Copy, scale=-1.0)

    state = {"prev_last_sin": None}

    def emit_exp(pair, sh, exp_insts):
        """Scores + exp + range-wraps + denominator for one chunk.

        Per-head data is packed at [:, hh*512:(hh+1)*512] of [128, 1024]
        SBUF tiles so downstream sins/muls run at N=1024.
        """
        ssl = slice(sh * 512, sh * 512 + 512)
        E_tiles, W_tiles = [], []
        r_ps = ppr.tile([128, 512], F32, name="r_ps", tag="ps_r")
        for tb2 in range(KT // 2):
            # E/W tiles span two t-blocks so the sin/mul stream runs at N=2048
            Eb = pE.tile([128, 2048], MM_DT, name="Eb", tag="E")
            Wb = pW.tile([128, 2048], MM_DT, name="Wb", tag="W")
            for half in range(2):
                tblk = tb2 * 2 + half
                tsl = slice(tblk * 128, tblk * 128 + 128)
                for hh in range(2):
                    head = pair * 2 + hh
                    ps_re = pmm.tile([128, 512], F32, name="ps_sre", tag="mm")
                    ps_ip = pmm.tile([128, 512], F32, name="ps_sip", tag="mm")
                    nc.tensor.matmul(ps_re, lhsT=k_s1[:, head, tsl],
                                     rhs=q_s[:, head, ssl],
                                     start=True, stop=True)
                    nc.tensor.matmul(ps_ip, lhsT=k_s2[:, head, tsl],
                                     rhs=q_s[:, head, ssl],
                                     start=True, stop=True)
                    qsl = slice(half * 1024 + hh * 512,
                                half * 1024 + hh * 512 + 512)
                    ei = nc.scalar.activation(Eb[:, qsl], ps_re, ACTF.Exp)
                    exp_insts.append(ei)
                    if state["prev_last_sin"] is not None:
                        _dep(state["prev_last_sin"], ei,
                             "act-table: exp after sins")
                    nc.vector.add_range_wrap(Wb[:, qsl], ps_ip, shift=PI / 4.0,
                                             bound=PI, period=2.0 * PI)
            E_tiles.append(Eb)
            W_tiles.append(Wb)
        # denominator ones-matmuls batched after the score stream so they
        # never stall the PE FIFO waiting on an exp. 3 accumulation chains at
        # col tile_positions 0/32/64 run on the PE (hh0 is split over two
        # t-halves summed on the vector engine afterwards).
        def esl(tblk, hh):
            # E/W are [128, 2048] per t-block pair: quarter for (tblk, hh)
            off = (tblk % 2) * 1024 + hh * 512
            return E_tiles[tblk // 2][:, off:off + 512]

        for tstep in range(4):
            nc.tensor.matmul(r_ps[0:1, :],
                             lhsT=ones_col, rhs=esl(tstep, 0),
                             start=(tstep == 0), stop=(tstep == 3))
            nc.tensor.matmul(r_ps[32:33, :],
                             lhsT=ones_col, rhs=esl(4 + tstep, 0),
                             start=(tstep == 0), stop=(tstep == 3))
            for thalf in range(2):
                tblk = thalf * 4 + tstep
                nc.tensor.matmul(r_ps[64:65, :],
                                 lhsT=ones_col, rhs=esl(tblk, 1),
                                 start=(tstep == 0 and thalf == 0),
                                 stop=(tstep == 3 and thalf == 1))
        # denominators -> fast reciprocal rows (broadcast via AP view later)
        rtmp = psm.tile([1, 512], F32, name="rtmp", tag="rrow")
        nc.scalar.copy(rtmp, r_ps[32:33, :])
        Rb = {}
        for hh in range(2):
            rrow = psm.tile([1, 512], F32, name="rrow", tag="rrow")
            if hh == 0:
                nc.vector.tensor_add(rrow, r_ps[0:1, :], rtmp)
            else:
                nc.vector.tensor_copy(rrow, r_ps[64:65, :])
            rb = psm.tile([1, 512], F32, name="rb", tag="rb")
            nc.vector.reciprocal_approx_fast(rb, rrow)
            rbb = psm.tile([128, 512], F32, name="rbb", tag="rbb")
            nc.gpsimd.partition_broadcast(rbb, rb)
            Rb[hh] = rbb
        return (E_tiles, W_tiles, Rb)

    def emit_sin(pair, sh, chunk_state, last_exp):
        ssl = slice(sh * 512, sh * 512 + 512)
        E_tiles, W_tiles, Rb = chunk_state
        at_ps = {hh: pat.tile([128, 512], F32, name="at_ps", tag="ps_at")
                 for hh in range(2)}
        for tb2 in range(KT // 2):
            Eb = E_tiles[tb2]
            Wb = W_tiles[tb2]
            cw = pbt.tile([128, 2048], MM_DT, name="cw", tag="sintmp")
            si1 = nc.scalar.activation(cw, Wb, ACTF.Sin, bias=pi4)   # cos(im)
            sw = pbt.tile([128, 2048], MM_DT, name="sw", tag="sintmp")
            si2 = nc.scalar.activation(sw, Wb, ACTF.Sin, bias=pi4n)  # sin(im)
            _dep(last_exp, si1, "act-table: sins after exps")
            _dep(last_exp, si2, "act-table: sins after exps")
            state["prev_last_sin"] = si2
            ar = pbt.tile([128, 2048], MM_DT, name="ar", tag="avr")
            nc.vector.tensor_mul(ar, Eb, cw)         # exp*cos
            ai = pbt.tile([128, 2048], MM_DT, name="ai", tag="avr")
            nc.vector.tensor_mul(ai, Eb, sw)         # exp*sin
            # stacked AV matmuls: comps (Vr,Vi) with rhs=ar, (-Vi,Vr) with ai
            for half in range(2):
                tblk = tb2 * 2 + half
                for hh in range(2):
                    head = pair * 2 + hh
                    qsl = slice(half * 1024 + hh * 512,
                                half * 1024 + hh * 512 + 512)
                    nc.tensor.matmul(at_ps[hh], lhsT=v[:, tblk, head, 1:3, :],
                                     rhs=ar[:, qsl],
                                     start=(tblk == 0), stop=False)
                    nc.tensor.matmul(at_ps[hh], lhsT=v[:, tblk, head, 0:2, :],
                                     rhs=ai[:, qsl],
                                     start=False, stop=(tblk == KT - 1))
        # normalize + copy out (stacked [re;im] rows share the same 1/r row)
        for hh in range(2):
            head = pair * 2 + hh
            nc.vector.tensor_mul(attn_s[:, head, ssl], at_ps[hh], Rb[hh])

    # =================== phase-C pools + output projection ===================
    pc_ctx = ExitStack()

    def open_phC():
        pc = pc_ctx.enter_context(tc.tile_pool(name="phC", bufs=1))
        po = pc_ctx.enter_context(tc.tile_pool(name="phC_o", bufs=4))
        wo = {}
        for sfx in ("S_re", "S_im"):
            wt = pc.tile([128, 4, S], MM_DT, name=f"wo{sfx}", tag=f"wo{sfx}")
            nc.sync.dma_start(
                out=wt, in_=ins["wo" + sfx])
            wo[sfx] = wt
        return pc, po, wo

    def emit_out(po, wo, sblks):
        for sblk in sblks:
            bsl = slice(sblk * 128, sblk * 128 + 128)
            for nt in range(2):
                nsl = slice(nt * 512, nt * 512 + 512)
                ps_or = pmm.tile([128, 512], F32, name="ps_or", tag="mm")
                ps_oi = pmm.tile([128, 512], F32, name="ps_oi", tag="mm")
                for h in range(4):  # contraction over heads (stacked comps)
                    la = attn_s[:, h, bsl]
                    nc.tensor.matmul(ps_or, lhsT=la, rhs=wo["S_re"][:, h, nsl],
                                     start=(h == 0), stop=(h == 3))
                    nc.tensor.matmul(ps_oi, lhsT=la, rhs=wo["S_im"][:, h, nsl],
                                     start=(h == 0), stop=(h == 3))
                o_r = po.tile([128, 512], MM_DT, name="o_r", tag="otmp")
                o_i = po.tile([128, 512], MM_DT, name="o_i", tag="otmp")
                nc.scalar.copy(o_r, ps_or)
                nc.vector.tensor_copy(o_i, ps_oi)
                nc.sync.dma_start(out=outs["out_re"][bsl, nsl], in_=o_r)
                nc.sync.dma_start(out=outs["out_im"][bsl, nsl], in_=o_i)

    # =================== pipelined emission ===================
    # Chunk order (0,0),(1,0) then out-proj for s<512, then (0,1),(1,1) and
    # the remaining out-proj: the s<512 output projection overlaps the second
    # half of attention instead of all trailing at the end. Per-chunk
    # exp/sin table phases (E S E S ...) are kept via explicit deps.
    emit_qk("wq", 0, "qc8", "qs8")
    emit_qk("wk", 0, "kcos", "ksin")
    wv = load_wv()
    emit_qk("wq", 1, "qc8", "qs8")      # independent PE work while the
    exps00 = []                          # chunk-(0,0) stack DMAs land
    st00 = emit_exp(0, 0, exps00)
    emit_qk("wk", 1, "kcos", "ksin")
    emit_v(wv)
    phA.close()  # release x/weights/tables space before phase C pools open

    pc, po, wo = open_phC()
    emit_sin(0, 0, st00, exps00[-1])
    exps10 = []
    st10 = emit_exp(1, 0, exps10)
    emit_sin(1, 0, st10, exps10[-1])
    emit_out(po, wo, range(0, 2))        # s < 512 needs only chunks (*,0)
    exps01 = []
    st01 = emit_exp(0, 1, exps01)
    emit_sin(0, 1, st01, exps01[-1])
    emit_out(po, wo, range(2, 4))
    exps11 = []
    st11 = emit_exp(1, 1, exps11)
    emit_sin(1, 1, st11, exps11[-1])
    emit_out(po, wo, range(4, 8))

    pc_ctx.close()
    pB.close()
    ctx.close()


_IN_SPECS = (
    [("xT_re", [D, S], MM_DT), ("xT_im", [D, S], MM_DT)]
    + [(w + sfx, [2, 128, KT, 128], MM_DT) for w in ("wq", "wk")
       for sfx in ("P_re", "P_im", "P_imn")]
    + [("wv" + sfx, [128, KT, JG], MM_DT) for sfx in ("P_re", "P_im", "P_imn")]
    + [("wo" + sfx, [128, 4, D], MM_DT) for sfx in ("S_re", "S_im")]
    + [(t, [128, S], TAB_DT) for t in ("qc8", "qs8", "kcos", "ksin")]
)


def build_program():
    nc = bacc.Bacc("TRN2", target_bir_lowering=False, debug=False,
                   enable_asserts=False, num_devices=8)
    ins = {name: nc.dram_tensor(name, shape, dt, kind="ExternalInput").ap()
           for name, shape, dt in _IN_SPECS}
    outs = {name: nc.dram_tensor(name, [S, D], MM_DT, kind="ExternalOutput").ap()
            for name in ("out_re", "out_im")}
    with tile.TileContext(nc) as tc:
        _build_kernel(tc, ins, outs)
    nc.compile()
    return nc


def _make_tables():
    inv_freq = 1.0 / (10000.0 ** (np.arange(DH, dtype=np.float64) / DH))
    ang = np.arange(S, dtype=np.float64)[:, None] * inv_freq[None, :]  # [S, DH]
    angT = ang.T  # [DH, S]
    ang128 = np.concatenate([angT, angT], axis=0)  # [128, S]
    c = np.cos(ang128)
    s = np.sin(ang128)
    tab_np = np.float16
    return {
        "qc8": (c * 0.125).astype(tab_np),
        "qs8": (s * 0.125).astype(tab_np),
        "kcos": c.astype(tab_np),
        "ksin": s.astype(tab_np),
    }


def _core_inputs(inputs, c, tables):
    b, g = divmod(c, 4)
    rows = slice(g * JG, (g + 1) * JG)

    def f(a):
        return np.ascontiguousarray(np.asarray(a, dtype=np.float32)).astype(MM_NP)

    woT_re = np.asarray(inputs["wo_re"], dtype=np.float32)[:, rows].T  # [256, D]
    woT_im = np.asarray(inputs["wo_im"], dtype=np.float32)[:, rows].T
    # per-head component stacks: [woT_re(h); -woT_im(h)] and [woT_im; woT_re]
    wsr = np.stack([np.concatenate([woT_re[h * 64:(h + 1) * 64],
                                    -woT_im[h * 64:(h + 1) * 64]], axis=0)
                    for h in range(4)], axis=1)  # [128, 4, D]
    wsi = np.stack([np.concatenate([woT_im[h * 64:(h + 1) * 64],
                                    woT_re[h * 64:(h + 1) * 64]], axis=0)
                    for h in range(4)], axis=1)
    m = {
        "xT_re": f(np.asarray(inputs["x_re"])[b].T),
        "xT_im": f(np.asarray(inputs["x_im"])[b].T),
        "woS_re": f(wsr),
        "woS_im": f(wsi),
    }
    def qk_pairs(wT):
        # [D, 256] -> [pair, p, kt, j]: contiguous per-pair DMA blocks
        return wT.reshape(KT, 128, 2, 128).transpose(2, 1, 0, 3)

    def v_blk(wT):
        # [D, 256] -> [p, kt, j]
        return wT.reshape(KT, 128, JG).transpose(1, 0, 2)

    for w, lay in (("wq", qk_pairs), ("wk", qk_pairs), ("wv", v_blk)):
        wre = np.asarray(inputs[w + "_re"], dtype=np.float32)[rows]
        wim = np.asarray(inputs[w + "_im"], dtype=np.float32)[rows]
        m[w + "P_re"] = f(lay(wre.T))
        m[w + "P_im"] = f(lay(wim.T))
        m[w + "P_imn"] = f(lay(-wim.T))
    m.update(tables)
    return m


_PROGRAM = None


def _get_program():
    global _PROGRAM
    if _PROGRAM is None:
        _PROGRAM = build_program()
    return _PROGRAM


def run(inputs, trace=False, **kwargs):
    nc = _get_program()
    tables = _make_tables()
    in_maps = [_core_inputs(inputs, c, tables) for c in range(8)]
    res = run_bass_kernel_spmd(nc, in_maps, list(range(8)), trace=trace, **kwargs)
    B = 2
    out = np.zeros((B, S, D, 2), np.float32)
    for c, r in enumerate(res.results):
        b = c // 4
        out[b, :, :, 0] += r["out_re"].astype(np.float32)
        out[b, :, :, 1] += r["out_im"].astype(np.float32)
    return out, res


def kernel(**inputs):
    out, _ = run(inputs)
    return out


if __name__ == "__main__":
    nc = build_program()
    print("program built + compiled OK")
